# revision 28
# baseline (speedup 1.0000x reference)
"""GGNN (gated graph NN) forward on 8 Trainium2 NeuronCores.

Strategy (node-partitioned, SPMD, column-phased pipeline):
  - Nodes are permuted and packed into 8 cores x 20 bins x 96 node-column
    slots such that, for every (bin, etype), the number of in-edges is <= 128.
    Aggregation is a fixed static structure: one 128-edge tile per
    (etype, bin).
  - Per-core node columns are laid out PHASE-PADDED: 4 phases x 512 PSUM-bank
    cols, each phase = 5 bins x 96 = 480 real cols + 32 pad.  Every bin
    window sits inside one PSUM bank, so scatter/W matmuls never split.
  - h is kept per-core transposed in SBUF ([128 hid, 2048 cols]); a row-major
    bf16 copy lives in DRAM (AllGathered each step, in 4 column chunks) and
    is the source for per-edge dma_gather.
  - Steps are processed phase-by-phase: after phase k's 13 etypes are
    gathered + scatter-summed + W-projected into aT bank k, the GRU chunk k,
    h-transpose, bf16 cast and DMA-out all run while phase k+1's gathers
    (the DMA bottleneck) proceed.  Only the last phase's GRU and the
    (single, full) AllGather remain exposed per step.
  - PSUM: aT (banks 0-3), B scatter double-buffer (banks 4-5), gates r/z
    (banks 6-7); gate ni reuses the freed aT bank k, nh reuses aT bank k-1.
  - Dead ends measured on this HW: chunked AllGathers lose (each collective
    pays a ~15-20us cross-core handshake, serialized on the CC cores);
    direct dma_scatter_add into the Shared table is silently per-chip (the
    8 cores span 4 chips; "Shared" DRAM is replicated per chip and only
    collective DMAs write all replicas).
"""

import numpy as np
import ml_dtypes

import concourse.bacc as bacc
import concourse.mybir as mybir
import concourse.tile as tile
from concourse.masks import make_identity
from concourse.bass_utils import run_bass_kernel_spmd

BF16_NP = ml_dtypes.bfloat16

F32 = mybir.dt.float32
F32R = mybir.dt.float32r
BF16 = mybir.dt.bfloat16
I16 = mybir.dt.int16
AF = mybir.ActivationFunctionType
OP = mybir.AluOpType

HID = 128
USE_F32R = True  # fp32r (tf32-ish) for W/GRU matmuls


class Cfg:
    def __init__(self, n_cores, n_etypes, n_steps, n_graphs, n_classes, in_dim):
        self.n_cores = n_cores
        self.T = n_etypes
        self.steps = n_steps
        self.G = n_graphs
        self.C = n_classes
        self.in_dim = in_dim
        self.bin = 96                       # node columns per bin
        self.bins = 20                      # bins per core
        self.phases = 4
        self.bpp = self.bins // self.phases  # bins per phase = 5
        self.pcols = 512                    # padded cols per phase (1 bank)
        self.rcols = self.bpp * self.bin    # real cols per phase = 480
        self.cols = self.phases * self.pcols  # padded cols per core = 2048
        self.shard = self.bins * self.bin   # real node slots per core = 1920
        self.ntok = n_cores * self.cols     # padded token space = 16384
        self.jt = self.cols // 128          # transpose tiles per core = 16
        self.scols = self.T * self.bins * self.bin  # S cols = 24960
        self.pbins = self.T * self.bpp      # bins per phase across etypes = 65
        self.idxc_ph = self.pbins * 8       # idx cols per phase = 520
        # gather call sizes (bins per call, <=8 to stay under Q7 ring limit)
        self.call_bins = [7, 7, 7, 7, 7, 7, 7, 8, 8]
        assert sum(self.call_bins) == self.pbins


CFG_FULL = dict(n_cores=8, n_etypes=13, n_steps=6, n_graphs=64, n_classes=10,
                in_dim=100)


# ---------------------------------------------------------------- host prep

def _pack_nodes(deg, cfg):
    """Assign each node to a (global bin, slot) s.t. per-(bin,etype) in-edge
    count <= 128 and per-bin node count <= cfg.bin. Returns bin_of, pos_of."""
    N = deg.shape[0]
    nbins = cfg.n_cores * cfg.bins
    assert N <= nbins * cfg.bin, "not enough node slots"
    used_e = np.zeros((nbins, cfg.T), np.int64)
    used_s = np.zeros(nbins, np.int64)
    order = np.lexsort((-deg.sum(1), -deg.max(1)))
    bin_of = np.empty(N, np.int64)
    for v in order:
        dv = deg[v]
        ok = (used_s < cfg.bin) & ((used_e + dv) <= 128).all(1)
        if not ok.any():
            raise RuntimeError("bin packing failed")
        cand = np.nonzero(ok)[0]
        load = (used_e[cand] + dv).max(1) * 1.0 + used_s[cand] * 0.01
        b = cand[np.argmin(load)]
        used_e[b] += dv
        bin_of[v] = b
        used_s[b] += 1
    pos_of = np.empty(N, np.int64)
    fill = np.zeros(nbins, np.int64)
    for v in range(N):
        b = bin_of[v]
        pos_of[v] = fill[b]
        fill[b] += 1
    return bin_of, pos_of


def make_plan(feat, src, dst, etypes, graph_ids, W_e, b_e, W_ih, W_hh, b_ih,
              b_hh, W_cls, b_cls, cfg):
    N = feat.shape[0]
    T, B_, BINS, NC = cfg.T, cfg.bin, cfg.bins, cfg.n_cores
    BPP, PC = cfg.bpp, cfg.pcols
    deg = np.zeros((N, T), np.int64)
    np.add.at(deg, (dst, etypes), 1)
    gbin_of, pos_of = _pack_nodes(deg, cfg)

    core_of = gbin_of // BINS
    lb_of = gbin_of % BINS
    # padded column within core: phase = lb//5, 96*(lb%5) + pos within phase
    pcol_of = PC * (lb_of // BPP) + B_ * (lb_of % BPP) + pos_of
    ptok_of = core_of * cfg.cols + pcol_of      # global padded token id

    # --- edge plan ---
    et = etypes.astype(np.int64)
    c_e = core_of[dst]
    lb_e = lb_of[dst]
    tile_e = et * BINS + lb_e                   # per-core tile 0..T*BINS-1
    order = np.lexsort((pos_of[dst], tile_e, c_e))
    c_o, tile_o = c_e[order], tile_e[order]
    pos_o = pos_of[dst][order]
    stok_o = ptok_of[src][order]
    key = c_o * (T * BINS) + tile_o
    boundaries = np.nonzero(np.diff(key))[0] + 1
    starts = np.concatenate([[0], boundaries])
    group_of = np.searchsorted(starts, np.arange(len(key)), side="right") - 1
    row = np.arange(len(key)) - starts[group_of]
    assert row.max() < 128, "edge cap exceeded (packing bug)"

    S_host = np.zeros((NC, 128, cfg.scols), np.float32)
    np.add.at(S_host, (c_o, row, tile_o * B_ + pos_o), 1.0)

    idx_lin = np.zeros((NC, T * BINS, 128), np.int64)
    idx_lin[c_o, tile_o, row] = stok_o
    # per-phase flat order: for t in 0..T-1, wb in 0..4: tile (t, 5*ph+wb)
    idx_host = np.zeros((NC, 16, cfg.phases * cfg.idxc_ph), np.int64)
    for ph in range(cfg.phases):
        blk = np.empty((NC, cfg.pbins, 128), np.int64)
        for t in range(T):
            for wb in range(BPP):
                blk[:, t * BPP + wb, :] = idx_lin[:, t * BINS + ph * BPP + wb, :]
        flat = blk.reshape(NC, cfg.pbins * 128)          # [NC, 8320]
        wrapped = flat.reshape(NC, cfg.idxc_ph, 16).transpose(0, 2, 1)
        idx_host[:, :, ph * cfg.idxc_ph:(ph + 1) * cfg.idxc_ph] = wrapped
    idx_host = np.tile(idx_host, (1, 8, 1)).astype(np.int16)

    # scatter indices: per phase, this core's 512 table rows, wrapped [16,32]
    sidx_host = np.zeros((NC, 16, cfg.phases * 32), np.int64)
    for c in range(NC):
        for ph in range(cfg.phases):
            rows = c * cfg.cols + ph * PC + np.arange(PC)
            sidx_host[c, :, ph * 32:(ph + 1) * 32] = rows.reshape(32, 16).T
    sidx_host = np.tile(sidx_host, (1, 8, 1)).astype(np.int16)

    # --- degree matrix (for b_e bias), per core [T, cols] padded ---
    D_host = np.zeros((NC, T, cfg.cols), np.float32)
    np.add.at(D_host, (c_e, et, pcol_of[dst]), 1.0)

    # --- graph one-hot, per core [128, jt*G] ---
    G_host = np.zeros((NC, 128, cfg.jt * cfg.G), np.float32)
    g_n = graph_ids[np.arange(N)]
    j_n = pcol_of // 128
    p_n = pcol_of % 128
    np.add.at(G_host, (core_of, p_n, j_n * cfg.G + g_n), 1.0)

    # --- h0 (padded token space) ---
    h0 = np.zeros((cfg.ntok, HID), np.float32)
    h0[ptok_of, :cfg.in_dim] = feat
    h0_pair = h0.astype(BF16_NP)
    h0T = np.zeros((NC, 128, cfg.cols), np.float32)
    for c in range(NC):
        h0T[c] = h0[c * cfg.cols:(c + 1) * cfg.cols].T

    # --- weights ---
    W_host = np.ascontiguousarray(W_e.transpose(1, 0, 2).reshape(128, T * HID))
    WihT = np.ascontiguousarray(W_ih.T)
    WhhT = np.ascontiguousarray(W_hh.T)
    bias4 = np.stack([
        b_ih[0:HID] + b_hh[0:HID],
        b_ih[HID:2 * HID] + b_hh[HID:2 * HID],
        b_ih[2 * HID:],
        b_hh[2 * HID:],
    ], axis=1).astype(np.float32)
    WclsT = np.ascontiguousarray(W_cls.T).astype(np.float32)
    bclsG = np.tile(b_cls[None, :], (cfg.G, 1)).astype(np.float32)

    in_maps = []
    for c in range(NC):
        in_maps.append({
            "h0_pair": h0_pair,
            "h0T": h0T[c],
            "S": S_host[c].astype(BF16_NP),
            "idx": idx_host[c],
            "sidx": sidx_host[c],
            "D": D_host[c],
            "G": G_host[c],
            "W": W_host.astype(np.float32),
            "Wih": WihT.astype(np.float32),
            "Whh": WhhT.astype(np.float32),
            "be": np.ascontiguousarray(b_e).astype(np.float32),
            "bias4": bias4,
            "Wcls": WclsT,
            "bcls": bclsG,
        })
    return in_maps


# ---------------------------------------------------------------- bass build

def build_nc(cfg):
    nc = bacc.Bacc("TRN2", target_bir_lowering=False, debug=False,
                   num_devices=cfg.n_cores, num_swdge_queues=4)
    T, BINS, B_, BPP = cfg.T, cfg.bins, cfg.bin, cfg.bpp
    PC, RC, COLS, JT, PH = cfg.pcols, cfg.rcols, cfg.cols, cfg.jt, cfg.phases

    d_pair0 = nc.dram_tensor("h0_pair", [cfg.ntok, HID], BF16, kind="ExternalInput")
    d_h0T = nc.dram_tensor("h0T", [128, COLS], F32, kind="ExternalInput")
    d_S = nc.dram_tensor("S", [128, cfg.scols], BF16, kind="ExternalInput")
    d_idx = nc.dram_tensor("idx", [128, PH * cfg.idxc_ph], I16, kind="ExternalInput")
    d_sidx = nc.dram_tensor("sidx", [128, PH * 32], I16, kind="ExternalInput")
    d_D = nc.dram_tensor("D", [T, COLS], F32, kind="ExternalInput")
    d_G = nc.dram_tensor("G", [128, JT * cfg.G], F32, kind="ExternalInput")
    d_W = nc.dram_tensor("W", [128, T * HID], F32, kind="ExternalInput")
    d_Wih = nc.dram_tensor("Wih", [128, 3 * HID], F32, kind="ExternalInput")
    d_Whh = nc.dram_tensor("Whh", [128, 3 * HID], F32, kind="ExternalInput")
    d_be = nc.dram_tensor("be", [T, HID], F32, kind="ExternalInput")
    d_bias4 = nc.dram_tensor("bias4", [128, 4], F32, kind="ExternalInput")
    d_Wcls = nc.dram_tensor("Wcls", [128, cfg.C], F32, kind="ExternalInput")
    d_bcls = nc.dram_tensor("bcls", [cfg.G, cfg.C], F32, kind="ExternalInput")
    d_out = nc.dram_tensor("out", [cfg.G, cfg.C], F32, kind="ExternalOutput")

    aspace = "Shared" if cfg.n_cores > 4 else "Local"
    cc_in = [nc.dram_tensor(f"cc_in{i}", [COLS, HID], BF16) for i in range(2)]
    # shared h tables (+1 barrier row); all cores gather from these
    tbl = [nc.dram_tensor(f"tbl{i}", [cfg.ntok + 1, HID], BF16,
                          addr_space=aspace) for i in range(2)]
    bar_in = nc.dram_tensor("bar_in", [1, HID], BF16)
    hg_in = nc.dram_tensor("hg_in", [cfg.G, HID], F32)
    hg_out = nc.dram_tensor("hg_out", [cfg.G, HID], F32, addr_space=aspace)

    MMDT = F32R if USE_F32R else F32

    with tile.TileContext(nc) as tc:
        def sb(name, shape, dt=F32):
            return nc.alloc_sbuf_tensor(name, list(shape), dt).ap()

        def ps(name, shape, dt=F32):
            return nc.alloc_psum_tensor(name, list(shape), dt).ap()

        S_sb = sb("S_sb", [128, cfg.scols], BF16)
        idx_sb = sb("idx_sb", [128, PH * cfg.idxc_ph], I16)
        sidx_sb = sb("sidx_sb", [128, PH * 32], I16)
        hist = [sb(f"hist{i}", [128, COLS], BF16) for i in range(2)]
        delta_sb = [sb(f"delta{i}", [128, PC], BF16) for i in range(2)]
        junk_sb = sb("junk_sb", [128, HID], BF16)
        pidx_sb = sb("pidx_sb", [128, 8], I16)
        hT = sb("hT", [128, COLS])
        aT_sb = sb("aT_sb", [128, COLS], MMDT)
        W_sb = sb("W_sb", [128, T * HID], MMDT)
        Wih_sb = sb("Wih_sb", [128, 3 * HID], MMDT)
        Whh_sb = sb("Whh_sb", [128, 3 * HID], MMDT)
        be_sb = sb("be_sb", [T, HID], MMDT)
        D_sb = sb("D_sb", [T, COLS], MMDT)
        bias_sb = sb("bias_sb", [128, 4])
        G_sb = sb("G_sb", [128, JT * cfg.G])
        Wcls_sb = sb("Wcls_sb", [128, cfg.C])
        bcls_sb = sb("bcls_sb", [cfg.G, cfg.C])
        ident = sb("ident", [128, 128])
        h_rows = sb("h_rows", [128, COLS])
        pair_sb = sb("pair_sb", [128, COLS], BF16)
        hg_sb = sb("hg_sb", [cfg.G, HID])
        hgT_sb = sb("hgT_sb", [128, cfg.G])
        out_sb = sb("out_sb", [cfg.G, cfg.C])
        hTr = sb("hTr", [128, COLS], F32R) if USE_F32R else None

        # phase-wide gather buffer, double-buffered across phases
        gbuf = [sb(f"gbuf{i}", [128, cfg.pbins * HID], BF16) for i in range(2)]
        Bsb = [sb(f"Bsb{i}", [128, RC], MMDT) for i in range(2)]
        gsc = [{nm: sb(f"gsc{i}_{nm}", [128, RC])
                for nm in ("r", "z", "hn", "t1", "t2", "n", "d1", "d2")}
               for i in range(2)]

        aT_ps = ps("aT_ps", [128, COLS])                     # banks 0-3
        B_ps = [ps(f"B_ps{i}", [128, 512]) for i in range(2)]  # banks 4-5
        g_ps = [ps(f"g_ps{i}", [128, 512]) for i in range(2)]  # banks 6-7

        # ---------------- setup loads ----------------
        if USE_F32R:
            stage = sb("stage", [128, T * HID])
            nc.sync.dma_start(stage[:], d_W[:])
            nc.vector.tensor_copy(W_sb[:], stage[:])
            stage2 = sb("stage2", [128, 3 * HID])
            nc.sync.dma_start(stage2[:], d_Wih[:])
            nc.vector.tensor_copy(Wih_sb[:], stage2[:])
            stage3 = sb("stage3", [128, 3 * HID])
            nc.sync.dma_start(stage3[:], d_Whh[:])
            nc.vector.tensor_copy(Whh_sb[:], stage3[:])
            stage4 = sb("stage4", [T, HID])
            nc.sync.dma_start(stage4[:], d_be[:])
            nc.vector.tensor_copy(be_sb[:], stage4[:])
            stage5 = sb("stage5", [T, COLS])
            nc.sync.dma_start(stage5[:], d_D[:])
            nc.vector.tensor_copy(D_sb[:], stage5[:])
        else:
            nc.sync.dma_start(W_sb[:], d_W[:])
            nc.sync.dma_start(Wih_sb[:], d_Wih[:])
            nc.sync.dma_start(Whh_sb[:], d_Whh[:])
            nc.sync.dma_start(be_sb[:], d_be[:])
            nc.sync.dma_start(D_sb[:], d_D[:])
        nc.sync.dma_start(idx_sb[:], d_idx[:])
        nc.sync.dma_start(sidx_sb[:], d_sidx[:])
        nc.gpsimd.memset(pidx_sb[:], 0)
        SC = BINS * B_
        for t in range(T):
            nc.sync.dma_start(S_sb[:, t * SC:(t + 1) * SC],
                              d_S[:, t * SC:(t + 1) * SC])
        nc.sync.dma_start(hT[:], d_h0T[:])
        nc.sync.dma_start(bias_sb[:], d_bias4[:])
        nc.sync.dma_start(G_sb[:], d_G[:])
        nc.sync.dma_start(Wcls_sb[:], d_Wcls[:])
        nc.sync.dma_start(bcls_sb[:], d_bcls[:])
        make_identity(nc, ident[:])

        # ---------------- steps ----------------
        gq = [0]  # rotating SWDGE queue
        for s in range(cfg.steps):
            pair_src = d_pair0 if s == 0 else tbl[s % 2]
            dst = cc_in[(s + 1) % 2]
            dst3 = dst[:].rearrange("(j p) d -> p j d", p=128)
            pr3 = pair_sb[:].rearrange("p (j d) -> p j d", d=HID)
            tout = tbl[(s + 1) % 2]
            hist_w = hist[(s + 1) % 2]
            # NOTE: direct dma_scatter_add into the Shared table does NOT
            # work across chips (8 cores = 4 chips x 2 NCs; "Shared" DRAM is
            # replicated per chip and only collectives write all replicas),
            # so every step's table write goes through the AllGather.
            use_ag = True

            def issue_scatter(kk):
                nc.gpsimd.dma_scatter_add(
                    tout[:],
                    delta_sb[kk % 2][:].rearrange("p (j d) -> p j d", d=HID),
                    sidx_sb[:, kk * 32:(kk + 1) * 32],
                    PC, PC, HID, queue_num=kk)

            for k in range(PH):
                c0 = k * PC
                # bias init: aT bank k = be^T @ D  (start resets whole bank)
                nc.tensor.matmul(aT_ps[:, c0:c0 + PC], be_sb[:],
                                 D_sb[:, c0:c0 + PC], start=True, stop=False)

                g = gbuf[k % 2]
                g3 = g[:].rearrange("p (b d) -> p b d", d=HID)
                b0 = 0
                for ci, nb in enumerate(cfg.call_bins):
                    nc.gpsimd.dma_gather(
                        g3[:, b0:b0 + nb, :], pair_src[:],
                        idx_sb[:, k * cfg.idxc_ph + b0 * 8:
                               k * cfg.idxc_ph + (b0 + nb) * 8],
                        nb * 128, nb * 128, HID,
                        queue_num=gq[0] % 4)
                    gq[0] += 1
                    b0 += nb
                    # previous phase's table scatter: slotted a few calls in
                    # so the in-order Q7 stream dispatches it as its delta
                    # lands, while the rings stay full of this phase's work
                    if s < cfg.steps - 1 and not use_ag and k >= 1 and ci == 3:
                        issue_scatter(k - 1)

                for t in range(T):
                    Bp = B_ps[t % 2]
                    for wb in range(BPP):
                        nc.tensor.matmul(
                            Bp[:, wb * B_:(wb + 1) * B_],
                            g3[:, t * BPP + wb, :],
                            S_sb[:, (t * BINS + k * BPP + wb) * B_:
                                 (t * BINS + k * BPP + wb + 1) * B_],
                            start=(wb == 0), stop=(wb == BPP - 1))
                    if t % 2:
                        nc.scalar.activation(Bsb[t % 2][:, :], Bp[:, 0:RC],
                                             AF.Identity)
                    else:
                        nc.vector.tensor_copy(Bsb[t % 2][:, :], Bp[:, 0:RC])
                    nc.tensor.matmul(aT_ps[:, c0:c0 + RC],
                                     W_sb[:, t * HID:(t + 1) * HID],
                                     Bsb[t % 2][:, :],
                                     start=False, stop=(t == T - 1))

                # ---------------- GRU chunk k ----------------
                nc.scalar.activation(aT_sb[:, c0:c0 + PC], aT_ps[:, c0:c0 + PC],
                                     AF.Identity)
                if USE_F32R:
                    nc.vector.tensor_copy(hTr[:, c0:c0 + RC], hT[:, c0:c0 + RC])

                r_ps = g_ps[0][:, 0:RC]
                z_ps = g_ps[1][:, 0:RC]
                ni_ps = aT_ps[:, c0:c0 + RC]
                nh_ps = aT_ps[:, ((k + 3) % 4) * PC:((k + 3) % 4) * PC + RC]
                a_c = aT_sb[:, c0:c0 + RC]
                h_c = (hTr if USE_F32R else hT)[:, c0:c0 + RC]
                nc.tensor.matmul(r_ps, Wih_sb[:, 0:HID], a_c, start=True, stop=False)
                nc.tensor.matmul(r_ps, Whh_sb[:, 0:HID], h_c, start=False, stop=True)
                nc.tensor.matmul(z_ps, Wih_sb[:, HID:2 * HID], a_c, start=True, stop=False)
                nc.tensor.matmul(z_ps, Whh_sb[:, HID:2 * HID], h_c, start=False, stop=True)
                nc.tensor.matmul(ni_ps, Wih_sb[:, 2 * HID:3 * HID], a_c, start=True, stop=True)
                nc.tensor.matmul(nh_ps, Whh_sb[:, 2 * HID:3 * HID], h_c, start=True, stop=True)

                sc = gsc[k % 2]
                nc.scalar.activation(sc["r"][:], r_ps, AF.Sigmoid, bias=bias_sb[:, 0:1])
                nc.scalar.activation(sc["z"][:], z_ps, AF.Sigmoid, bias=bias_sb[:, 1:2])
                nc.scalar.activation(sc["hn"][:], nh_ps, AF.Identity, bias=bias_sb[:, 3:4])
                nc.vector.tensor_tensor(out=sc["t1"][:], in0=sc["r"][:], in1=sc["hn"][:], op=OP.mult)
                nc.vector.tensor_tensor(out=sc["t2"][:], in0=sc["t1"][:], in1=ni_ps, op=OP.add)
                nc.scalar.activation(sc["n"][:], sc["t2"][:], AF.Tanh, bias=bias_sb[:, 2:3])
                nc.vector.tensor_tensor(out=sc["d1"][:], in0=hT[:, c0:c0 + RC], in1=sc["n"][:], op=OP.subtract)
                nc.vector.tensor_tensor(out=sc["d2"][:], in0=sc["d1"][:], in1=sc["z"][:], op=OP.mult)
                nc.vector.tensor_tensor(out=hT[:, c0:c0 + RC], in0=sc["d2"][:], in1=sc["n"][:], op=OP.add)

                # ------- transpose hT chunk -> rows; cast; DMA; AllGather ---
                for jj in range(4):
                    j = 4 * k + jj
                    tp = g_ps[0][:, jj * 128:(jj + 1) * 128]
                    nc.tensor.transpose(tp, hT[:, j * 128:(j + 1) * 128], ident[:])
                    if jj % 2:
                        nc.scalar.activation(h_rows[:, j * 128:(j + 1) * 128], tp, AF.Identity)
                    else:
                        nc.vector.tensor_copy(h_rows[:, j * 128:(j + 1) * 128], tp)
                if s < cfg.steps - 1:
                    nc.scalar.activation(pr3[:, 4 * k:4 * k + 4, :],
                                         h_rows[:].rearrange("p (j d) -> p j d", d=HID)[:, 4 * k:4 * k + 4, :],
                                         AF.Identity)
                    if use_ag:
                        nc.sync.dma_start(dst3[:, 4 * k:4 * k + 4, :],
                                          pr3[:, 4 * k:4 * k + 4, :])
                        if k == PH - 1:
                            nc.gpsimd.collective_compute(
                                "AllGather", OP.bypass,
                                ins=[dst[:]], outs=[tout[0:cfg.ntok, :]],
                                replica_groups=[list(range(cfg.n_cores))])
                    else:
                        # delta vs what the table currently holds (written 2
                        # steps ago); scatter-add makes it the new h rows
                        nc.vector.tensor_tensor(
                            out=delta_sb[k % 2][:, :],
                            in0=pair_sb[:, c0:c0 + PC],
                            in1=hist_w[:, c0:c0 + PC], op=OP.subtract)
                    nc.vector.tensor_copy(hist_w[:, c0:c0 + PC],
                                          pair_sb[:, c0:c0 + PC])
                    if not use_ag and k == PH - 1:
                        issue_scatter(k)
                        # barrier: a 256B AllReduce into the table's extra
                        # row. Its WAW on the scatters' (full-tensor-range)
                        # writes orders it after all 4 scatter completions;
                        # next step's gather in_ap covers the extra row, so
                        # peers' gathers wait for everyone's scatters.
                        nc.gpsimd.collective_compute(
                            "AllReduce", OP.add, ins=[bar_in[:]],
                            outs=[tout[cfg.ntok:cfg.ntok + 1, :]],
                            replica_groups=[list(range(cfg.n_cores))])

        # ---------------- readout ----------------
        hg_ps = B_ps[0][0:cfg.G, 0:HID]
        for j in range(JT):
            nc.tensor.matmul(hg_ps, G_sb[:, j * cfg.G:(j + 1) * cfg.G],
                             h_rows[:, j * 128:(j + 1) * 128],
                             start=(j == 0), stop=(j == JT - 1))
        nc.scalar.activation(hg_sb[:], hg_ps, AF.Identity)
        nc.sync.dma_start(hg_in[:], hg_sb[:])
        nc.gpsimd.collective_compute(
            "AllReduce", OP.add, ins=[hg_in[:]], outs=[hg_out[:]],
            replica_groups=[list(range(cfg.n_cores))])
        hg_all = sb("hg_all", [cfg.G, HID])
        nc.sync.dma_start(hg_all[:], hg_out[:])
        tp_ps = B_ps[1][:, 0:cfg.G]
        nc.tensor.transpose(tp_ps, hg_all[:], ident[0:cfg.G, 0:cfg.G])
        nc.vector.tensor_copy(hgT_sb[:], tp_ps)
        lg_ps = g_ps[1][0:cfg.G, 0:cfg.C]
        nc.tensor.matmul(lg_ps, hgT_sb[:], Wcls_sb[:], start=True, stop=True)
        nc.vector.tensor_tensor(out=out_sb[:], in0=lg_ps, in1=bcls_sb[:], op=OP.add)
        nc.sync.dma_start(d_out[:], out_sb[:])

    nc.compile()
    return nc


# ---------------------------------------------------------------- entry

_CACHE = {}
LAST_EXEC_NS = None
LAST_RESULTS = None
PROFILE = False


def _get_nc(cfg_key, cfg):
    if cfg_key not in _CACHE:
        _CACHE[cfg_key] = build_nc(cfg)
    return _CACHE[cfg_key]


def kernel(feat, src, dst, etypes, graph_ids, W_e, b_e, W_ih, W_hh, b_ih,
           b_hh, W_cls, b_cls):
    feat = np.asarray(feat, np.float32)
    args = dict(src=np.asarray(src), dst=np.asarray(dst),
                etypes=np.asarray(etypes), graph_ids=np.asarray(graph_ids),
                W_e=np.asarray(W_e, np.float32), b_e=np.asarray(b_e, np.float32),
                W_ih=np.asarray(W_ih, np.float32), W_hh=np.asarray(W_hh, np.float32),
                b_ih=np.asarray(b_ih, np.float32), b_hh=np.asarray(b_hh, np.float32),
                W_cls=np.asarray(W_cls, np.float32), b_cls=np.asarray(b_cls, np.float32))
    cfg = Cfg(**CFG_FULL)
    in_maps = make_plan(feat=feat, cfg=cfg, **args)
    nc = _get_nc("full", cfg)
    res = run_bass_kernel_spmd(nc, in_maps, list(range(cfg.n_cores)),
                               trace=PROFILE)
    global LAST_EXEC_NS, LAST_RESULTS
    LAST_EXEC_NS = res.exec_time_ns
    LAST_RESULTS = res
    return np.asarray(res.results[0]["out"], np.float32)
